# revision 3
# baseline (speedup 1.0000x reference)
"""GraphConv 2-layer GNN on 8 Trainium2 NeuronCores — fused fp8 edition.

Destination nodes are partitioned across the 8 cores (12500 each), sorted by
degree.  The host gathers per-edge source features into a feature-major fp8
payload [128, L*128]; the device streams it and fuses aggregation with the
W_rel matmul: PSUM accumulates W_rel @ x_src over each window's edge slots on
top of W_root @ x_dst (one 512-wide root matmul per 4-window quad), i.e.
exact fp32 accumulation of W@(sum x_j) + Wroot@x_dst.  A third of each
window's edge slots are pre-summed pairwise on the Vector engine (exact in
bf16) so the PE streams ~2/3 of the columns.  Layer-1 epilogue is a
single relu+bias activation per quad; layer-2 stages logits and runs one
whole-layer log_softmax chain.  Outputs stay staged in SBUF and are written
in large chunks.  Two SPMD launches with a host re-gather of the fp8 hidden
state between them.
"""
import sys
sys.path.insert(0, "/opt/trn_rl_repo")
import numpy as np
import ml_dtypes

import concourse.bacc as bacc
import concourse.mybir as mybir
import concourse.tile as tile
from concourse.bass_utils import run_bass_kernel_spmd
from concourse.masks import make_identity

BF16 = ml_dtypes.bfloat16
F8 = ml_dtypes.float8_e4m3
N, E, F, H, C = 100000, 1600000, 128, 128, 40
NCORES = 8
OWN = N // NCORES          # 12500 dst nodes per core
P = 128
NWIN = (OWN + P - 1) // P  # 98 windows of 128 dst lanes
OWNP = NWIN * P            # 12544
QUAD = 4                   # windows per PSUM tile (512 free dim)
NQUAD = (NWIN + QUAD - 1) // QUAD  # 25 (last quad has 2 windows)

BF = mybir.dt.bfloat16
F32 = mybir.dt.float32
FP8 = mybir.dt.float8e4

BIAS_REP = True
_graph_cache = {}


def _prep_graph(edge_index):
    """Host-side plan: per-core degree-sorted dst order, common window depth
    profile Dw, slot offsets, per-core payload column->src maps."""
    src = np.asarray(edge_index[0], dtype=np.int64)
    dst = np.asarray(edge_index[1], dtype=np.int64)
    deg = np.bincount(dst, minlength=N)
    orders = []
    for c in range(NCORES):
        ids = np.arange(c * OWN, (c + 1) * OWN)
        orders.append(ids[np.argsort(-deg[ids], kind="stable")])
    degs_sorted = np.stack([deg[o] for o in orders])  # [8, OWN]
    pad = np.zeros((NCORES, OWNP - OWN), np.int64)
    degs_sorted = np.concatenate([degs_sorted, pad], axis=1)
    Dw = []
    for w in range(NWIN):
        seg = degs_sorted[:, w * P:(w + 1) * P]
        Dw.append(max(1, int(seg.max())))
    offs = np.zeros(NWIN + 1, np.int64)
    offs[1:] = np.cumsum(Dw)
    L = int(offs[-1])

    core = dst // OWN
    colmaps = []
    for c in range(NCORES):
        rank_of = np.empty(OWN, np.int64)
        rank_of[orders[c] - c * OWN] = np.arange(OWN)
        m = core == c
        s_c, d_c = src[m], dst[m]
        r = rank_of[d_c - c * OWN]
        order = np.argsort(r, kind="stable")
        r_s = r[order]
        first = np.searchsorted(r_s, r_s)
        j = np.arange(len(r_s)) - first       # occurrence index within dst
        win = r_s // P
        lane = r_s % P
        cols = np.full((L, P), N, np.int64)   # N = zero-row sentinel
        cols[offs[win] + j, lane] = s_c[order]
        colmaps.append(cols.reshape(-1))      # payload col -> src id
    return orders, Dw, offs, colmaps, L


def _quads(Dw):
    """(w0, nwins, dsum) per quad of windows."""
    out = []
    for q in range(NQUAD):
        w0 = q * QUAD
        ng = min(QUAD, NWIN - w0)
        out.append((w0, ng, sum(int(d) for d in Dw[w0:w0 + ng])))
    return out


def _pairs(D):
    return int(D) // 2


def _build(layer, Dw, offs, R=1, timing=False, mode="full", psbufs=6):
    """Fused SPMD Bass program for one GraphConv layer.
    mode: 'full' | 'dma' (skip compute) | 'pe' (skip payload/xr DMA)."""
    FO = H if layer == 1 else C
    L = int(offs[-1])
    NCOL = L * P
    quads = _quads(Dw)
    DMAX = max(d for (_, _, d) in quads)
    PBMAX = max(sum(_pairs(Dw[w0 + i]) for i in range(nw))
                for (w0, nw, _) in quads)
    PBMAX = max(PBMAX, 1)
    nc = bacc.Bacc()
    if timing:
        xe = nc.dram_tensor("xe", [P, NCOL], FP8)
        xr = nc.dram_tensor("xr", [P, OWNP], FP8)
    else:
        xe = nc.declare_dram_parameter("xe", [P, NCOL], FP8, isOutput=False)
        xr = nc.declare_dram_parameter("xr", [P, OWNP], FP8, isOutput=False)
    wrel = nc.declare_dram_parameter("wrel", [F, FO], BF, isOutput=False)
    wroot = nc.declare_dram_parameter("wroot", [F, FO], BF, isOutput=False)
    bias = nc.declare_dram_parameter(
        "bias", [P, 1] if layer == 1 else [P, C], F32, isOutput=False)
    if mode == "dma" or layer == 1:
        out = nc.declare_dram_parameter("out", [P, OWNP], FP8, isOutput=True)
    else:
        out = nc.declare_dram_parameter("out", [P, NWIN * C], BF, isOutput=True)

    # flush staged layer-1 output every ~8 quads (32 windows)
    flush_after = set()
    for qi in range(NQUAD):
        if (qi + 1) % 8 == 0 or qi == NQUAD - 1:
            flush_after.add(qi)

    with tile.TileContext(nc) as tc:
        with (
            tc.tile_pool(name="const", bufs=1) as cpool,
            tc.tile_pool(name="stream", bufs=3) as spool,
            tc.tile_pool(name="work", bufs=3) as epool,
            tc.tile_pool(name="ps", bufs=psbufs, space="PSUM") as ppool,
        ):
            wrel_t = cpool.tile([F, FO], BF)
            nc.sync.dma_start(out=wrel_t[:], in_=wrel[:])
            wroot_t = cpool.tile([F, FO], BF)
            nc.sync.dma_start(out=wroot_t[:], in_=wroot[:])
            bias_t = cpool.tile([P, 1] if layer == 1 else [P, C], F32)
            nc.sync.dma_start(out=bias_t[:], in_=bias[:])
            if layer == 2:
                sstage = cpool.tile([P, NWIN, C], F32)
                extile = cpool.tile([P, NWIN, C], F32)
                mx = cpool.tile([P, NWIN, 1], F32)
                sm = cpool.tile([P, NWIN, 1], F32)
                ls = cpool.tile([P, NWIN, 1], F32)
                tot = cpool.tile([P, NWIN, 1], F32)
            xr_t = cpool.tile([P, OWNP], FP8)
            if layer == 1:
                ostage = cpool.tile([P, OWNP], FP8)
            else:
                ostage = cpool.tile([P, NWIN, C], BF)
            if mode == "pe":
                st_fixed = [cpool.tile([P, DMAX * P], FP8, name=f"stf{i}")
                            for i in range(3)]
                for t in st_fixed:
                    nc.vector.memset(t[:], 0)
                nc.vector.memset(xr_t[:], 0)

            XRCH = 8 * QUAD * P  # xr chunk: 8 quads of columns

            def softmax_range(wa, wb):
                n = wb - wa
                nc.vector.reduce_max(out=mx[:, wa:wb, :],
                                     in_=sstage[:, wa:wb, :],
                                     axis=mybir.AxisListType.X)
                nc.vector.tensor_tensor(
                    out=extile[:, wa:wb, :], in0=sstage[:, wa:wb, :],
                    in1=mx[:, wa:wb, :].to_broadcast([P, n, C]),
                    op=mybir.AluOpType.subtract)
                nc.scalar.activation(
                    out=extile[:, wa:wb, :], in_=extile[:, wa:wb, :],
                    func=mybir.ActivationFunctionType.Exp)
                nc.vector.reduce_sum(out=sm[:, wa:wb, :],
                                     in_=extile[:, wa:wb, :],
                                     axis=mybir.AxisListType.X)
                nc.scalar.activation(
                    out=ls[:, wa:wb, :], in_=sm[:, wa:wb, :],
                    func=mybir.ActivationFunctionType.Ln)
                nc.vector.tensor_add(out=tot[:, wa:wb, :],
                                     in0=mx[:, wa:wb, :], in1=ls[:, wa:wb, :])
                nc.vector.tensor_tensor(
                    out=ostage[:, wa:wb, :], in0=sstage[:, wa:wb, :],
                    in1=tot[:, wa:wb, :].to_broadcast([P, n, C]),
                    op=mybir.AluOpType.subtract)
                nc.sync.dma_start(out=out[:, wa * C:wb * C],
                                  in_=ostage[:, wa:wb, :])

            def body(_iv=None):
                if mode != "pe":
                    nc.sync.dma_start(out=xr_t[:, :XRCH], in_=xr[:, :XRCH])
                flush_from = 0
                for qi, (w0, nw, dsum) in enumerate(quads):
                    base = int(offs[w0]) * P
                    nf = nw * P  # free columns in this quad (512 or 256)
                    if mode != "pe" and qi % 8 == 0 and qi > 0:
                        c0 = qi * QUAD * P
                        c1 = min(c0 + XRCH, OWNP)
                        if c0 < OWNP:
                            nc.sync.dma_start(out=xr_t[:, c0:c1],
                                              in_=xr[:, c0:c1])
                    if mode == "pe":
                        st = st_fixed[qi % 3]
                    else:
                        st = spool.tile([P, DMAX * P], FP8, tag="stream")
                        nc.sync.dma_start(
                            out=st[:, :dsum * P],
                            in_=xe[:, base:base + dsum * P],
                        )
                    if mode == "dma":
                        if qi in flush_after:
                            wend = w0 + nw
                            ncols = min((wend - flush_from) * P, dsum * P)
                            nc.sync.dma_start(
                                out=out[:, flush_from * P:flush_from * P + ncols],
                                in_=st[:, :ncols])
                            flush_from = wend
                        continue
                    pb = epool.tile([P, PBMAX * P], BF, tag="pairbuf")
                    loc = 0
                    pbloc = 0
                    pbstarts = []
                    for wi in range(nw):
                        D = int(Dw[w0 + wi])
                        npair = _pairs(D)
                        pbstarts.append(pbloc)
                        if npair:
                            s0 = loc + D - 2 * npair
                            pv = st[:, s0 * P:(s0 + 2 * npair) * P].rearrange(
                                "p (k two f) -> p k two f", two=2, f=P)
                            ov = pb[:, pbloc * P:(pbloc + npair) * P].rearrange(
                                "p (k f) -> p k f", f=P)
                            nc.vector.tensor_tensor(
                                out=ov, in0=pv[:, :, 0, :], in1=pv[:, :, 1, :],
                                op=mybir.AluOpType.add)
                            pbloc += npair
                        loc += D
                    if layer == 1:
                        ps = ppool.tile([FO, QUAD * P], F32, tag="agg")
                        nc.tensor.matmul(
                            out=ps[:, :nf],
                            lhsT=wroot_t[:],
                            rhs=xr_t[:, w0 * P:w0 * P + nf],
                            start=True, stop=False, skip_group_check=True,
                        )
                        loc = 0
                        for wi in range(nw):
                            D = int(Dw[w0 + wi])
                            npair = _pairs(D)
                            ndirect = D - 2 * npair
                            for j in range(ndirect):
                                nc.tensor.matmul(
                                    out=ps[:, wi * P:(wi + 1) * P],
                                    lhsT=wrel_t[:],
                                    rhs=st[:, (loc + j) * P:(loc + j + 1) * P],
                                    start=False,
                                    stop=(npair == 0 and j == ndirect - 1),
                                    skip_group_check=True,
                                )
                            for k in range(npair):
                                kk = pbstarts[wi] + k
                                nc.tensor.matmul(
                                    out=ps[:, wi * P:(wi + 1) * P],
                                    lhsT=wrel_t[:],
                                    rhs=pb[:, kk * P:(kk + 1) * P],
                                    start=False,
                                    stop=(k == npair - 1),
                                    skip_group_check=True,
                                )
                            loc += D
                        nc.scalar.activation(
                            out=ostage[:, w0 * P:w0 * P + nf], in_=ps[:, :nf],
                            func=mybir.ActivationFunctionType.Relu,
                            bias=bias_t[:, :1], scale=1.0,
                        )
                        if qi in flush_after:
                            wend = w0 + nw
                            nc.sync.dma_start(
                                out=out[:, flush_from * P:wend * P],
                                in_=ostage[:, flush_from * P:wend * P],
                            )
                            flush_from = wend
                    else:
                        # flipped: slot tiles stationary, W [F, C] moving;
                        # PSUM directly accumulates [dst, class] per window.
                        loc = 0
                        for wi in range(nw):
                            D = int(Dw[w0 + wi])
                            npair = _pairs(D)
                            ndirect = D - 2 * npair
                            ps2 = ppool.tile([P, C], F32, tag="agg")
                            nc.tensor.matmul(
                                out=ps2[:],
                                lhsT=xr_t[:, (w0 + wi) * P:(w0 + wi + 1) * P],
                                rhs=wroot_t[:],
                                start=True, stop=False, skip_group_check=True,
                            )
                            for j in range(ndirect):
                                nc.tensor.matmul(
                                    out=ps2[:],
                                    lhsT=st[:, (loc + j) * P:(loc + j + 1) * P],
                                    rhs=wrel_t[:],
                                    start=False,
                                    stop=(npair == 0 and j == ndirect - 1),
                                    skip_group_check=True,
                                )
                            for k in range(npair):
                                kk = pbstarts[wi] + k
                                nc.tensor.matmul(
                                    out=ps2[:],
                                    lhsT=pb[:, kk * P:(kk + 1) * P],
                                    rhs=wrel_t[:],
                                    start=False,
                                    stop=(k == npair - 1),
                                    skip_group_check=True,
                                )
                            loc += D
                            nc.vector.tensor_add(
                                out=sstage[:, w0 + wi, :], in0=ps2[:],
                                in1=bias_t[:, :C])
                        if qi == 15:
                            softmax_range(0, 16 * QUAD)
                if layer == 2 and mode != "dma":
                    softmax_range(16 * QUAD, NWIN)

            if R > 1:
                with tc.For_i(0, R, 1):
                    body()
            else:
                body()

    nc.finalize()
    return nc


def _prep_table(table_f32):
    """table [N, F] float32 -> fp8 feature-major ext [F, N+1] (col N zeros)."""
    t8 = np.zeros((F, N + 1), F8)
    t8[:, :N] = table_f32.T.astype(F8)
    return t8


def _layer_inputs(t8, orders, colmaps, w_rel, w_root, b):
    FO = w_rel.shape[0]
    wrelT = np.ascontiguousarray(np.asarray(w_rel, np.float32).T).astype(BF16)
    wrootT = np.ascontiguousarray(np.asarray(w_root, np.float32).T).astype(BF16)
    if FO == H:
        bias = np.zeros((P, 1), np.float32)
        bias[:FO, 0] = np.asarray(b, np.float32)
    else:
        bias = np.tile(np.asarray(b, np.float32)[None, :], (P, 1))
    in_maps = []
    for c in range(NCORES):
        xe = np.take(t8, colmaps[c], axis=1)
        ordp = np.concatenate([orders[c], np.full(OWNP - OWN, N, np.int64)])
        xr = np.ascontiguousarray(np.take(t8, ordp, axis=1))
        in_maps.append({"xe": xe, "xr": xr,
                        "wrel": wrelT, "wroot": wrootT, "bias": bias})
    return in_maps


def kernel(x, edge_index, W1_rel, b1, W1_root, W2_rel, b2, W2_root, _R=1):
    x = np.asarray(x, np.float32)
    key = id(edge_index)
    if key not in _graph_cache:
        _graph_cache.clear()
        _graph_cache[key] = _prep_graph(edge_index)
    orders, Dw, offs, colmaps, L = _graph_cache[key]

    nc1 = _build(1, Dw, offs, R=_R)
    nc2 = _build(2, Dw, offs, R=_R)

    t8 = _prep_table(x)
    in1 = _layer_inputs(t8, orders, colmaps, W1_rel, W1_root, b1)
    res1 = run_bass_kernel_spmd(nc1, in1, list(range(NCORES)))
    t8h = np.zeros((F, N + 1), F8)
    for c in range(NCORES):
        t8h[:, orders[c]] = res1.results[c]["out"][:, :OWN]

    in2 = _layer_inputs(t8h, orders, colmaps, W2_rel, W2_root, b2)
    res2 = run_bass_kernel_spmd(nc2, in2, list(range(NCORES)))
    out = np.zeros((N, C), np.float32)
    for c in range(NCORES):
        o = res2.results[c]["out"].astype(np.float32).reshape(P, NWIN, C)
        out[orders[c]] = o.transpose(1, 0, 2).reshape(OWNP, C)[:OWN]
    return out
